# revision 1
# baseline (speedup 1.0000x reference)
"""Trainium2 Bass kernel for MixformerAttention (sparse attention) — v2.

Problem shape (hardcoded):
  x [B=64, N=320, C=768], W_qkv [768, 2304], W_proj [768, 768], b_proj [768]
  H=12 heads, Dh=64, template L=64, search=256. DP over batch on 8 cores.

v2 redesign vs baseline (empirically driven by the NTFF trace):
  * Every matmul whose lhsT had only 64 partition rows (scores, template,
    PV tail) paid a ~100ns serialized LDWEIGHTS. All attention operands are
    now zero-padded to full 128 contraction rows:
      - qTp: per-head q tiles [128, tok], data in the head's native 64-row
        half, zeros in the other half (DMA'd from the qk psum drain).
      - kT stays packed 2-heads/chunk; the junk half multiplies the zero
        half of qTp, contributing 0.
      - es/esm/va key-padded the same way (persistent tiles, pads zeroed
        once at startup).
  * attn^T computed with regular matmuls (lhsT=attn chunk, rhs=identity):
    weight-load overlaps, unlike transpose-mode where the data IS the
    weight load. Template/search chunks overlap-packed into one psum strip
    so each fc drains with a single copy.
  * Software-pipelined emission: scores/exp of batch g interleave with
    PV/attnT/proj of batch g-1 (deferred thunks), so the Act-engine exp
    latency never stalls the PE.
  * PSUM budget exactly 8 banks: pool_g 3x[128,768] + pool_s 1x[128,1024].
"""

import contextlib
import functools

import numpy as np

import concourse.bacc as bacc
import concourse.mybir as mybir
from concourse.bass_utils import run_bass_kernel_spmd
from concourse.masks import make_identity
from concourse.tile import TileContext

F32 = mybir.dt.float32
F16 = mybir.dt.float16

NCORES = 8
B, N, C = 64, 320, 768
H, DH = 12, 64
KS = C // 128  # 6 contraction subtiles
B_CORE = B // NCORES  # 8
PAIR_TOK = 2 * N  # 640
NPAIR = B_CORE // 2  # 4
TOK_CORE = B_CORE * N  # 2560
SLOT = 85  # psum col stride per head in PV output (6 heads in 510 cols)

KT_CHUNKS = [(0, 128), (128, 128), (256, 64)]  # key chunks per batch
P_CHUNKS = [(0, 128), (128, 128), (256, 64)]  # proj token chunks per batch


def build_kernel():
    nc = bacc.Bacc("TRN2", target_bir_lowering=False)
    x_t = nc.dram_tensor("xT16", [C, TOK_CORE], F16, kind="ExternalInput")
    wqkv_t = nc.dram_tensor("W_qkv16", [C, 3 * C], F16, kind="ExternalInput")
    wproj_t = nc.dram_tensor("W_proj16", [C, C], F16, kind="ExternalInput")
    bias_t = nc.dram_tensor("b_proj", [C], F32, kind="ExternalInput")
    out_t = nc.dram_tensor("out", [TOK_CORE, C], F16, kind="ExternalOutput")
    x_ap, out_ap = x_t.ap(), out_t.ap()

    with TileContext(nc) as tc:
        with contextlib.ExitStack() as ctx:
            P = {
                "const": ctx.enter_context(tc.tile_pool(name="const", bufs=1)),
                "stagep": ctx.enter_context(tc.tile_pool(name="stagep", bufs=1)),
                "xT": ctx.enter_context(tc.tile_pool(name="xT", bufs=2)),
                "qkfc": ctx.enter_context(tc.tile_pool(name="qkfc", bufs=3)),
                "outst": ctx.enter_context(tc.tile_pool(name="outst", bufs=2)),
                "rcp": ctx.enter_context(tc.tile_pool(name="rcp", bufs=4)),
                "pg": ctx.enter_context(tc.tile_pool(name="pg", bufs=2, space="PSUM")),
                "ps": ctx.enter_context(tc.tile_pool(name="ps", bufs=2, space="PSUM")),
            }
            const = P["const"]

            # ---- persistent constants ----
            wqkv16 = const.tile([128, KS, 3 * C], F16, tag="wqkv16")
            wproj16 = const.tile([128, KS, C], F16, tag="wproj16")
            bias_bc = const.tile([128, C], F32, tag="bias_bc")
            ident32 = const.tile([128, 128], F32, tag="ident32")
            ident16 = const.tile([128, 128], F16, tag="ident16")
            make_identity(nc, ident32)
            make_identity(nc, ident16)

            # ---- persistent double-slotted activation tiles ----
            # per-head padded q (slot = pair parity)
            qTp = const.tile([128, 2, H, PAIR_TOK], F16, tag="qTp")
            # packed kT feature chunks (2 heads per chunk)
            kTpk = const.tile([128, 2, KS, PAIR_TOK], F16, tag="kTpk")
            # v natural with ones column (slot = batch parity)
            va = const.tile([128, 2, 3, H, 66], F16, tag="va")
            # exp(scores) for search queries [key, h, q]  (256 = search q)
            es = const.tile([128, 2, 3, H, 256], F16, tag="es")
            # exp(scores) template [key<=64 padded, h, q0:64]
            esm = const.tile([128, 2, H, 64], F16, tag="esm")
            # attention rows (template 64 padded | search 128 | search 128)
            attn = const.tile([128, 2, 3, C], F16, tag="attn")
            # attn^T per batch [C-part, tok]
            attnT = const.tile([128, 2, KS, N], F16, tag="attnT")

            def emit_weight_load():
                # fp16 weights land directly in their SBUF tiles (host-cast),
                # column-sliced in the order the qk chains consume them
                for ks in range(KS):
                    nc.sync.dma_start(
                        wqkv16[:, ks, :], wqkv_t.ap()[ks * 128 : (ks + 1) * 128, :]
                    )
                for ks in range(KS):
                    nc.sync.dma_start(
                        wproj16[:, ks, :], wproj_t.ap()[ks * 128 : (ks + 1) * 128, :]
                    )
                brow = P["stagep"].tile([128, C], F32, tag="stagep")
                nc.sync.dma_start(brow[0:1, 0:C], bias_t.ap().unsqueeze(0))
                nc.gpsimd.partition_broadcast(bias_bc[:, :], brow[0:1, 0:C])

            def emit_pads():
                # one-time pad zeroing (emitted after A(0) so the DVE queue
                # drains xT first; WAW deps keep correctness)
                nc.gpsimd.memset(qTp[64:128, :, 0:H:2, :], 0.0)  # even heads
                nc.vector.memset(qTp[0:64, :, 1:H:2, :], 0.0)  # odd heads
                nc.vector.memset(es[64:128, :, 2, :, :], 0.0)  # key chunk 2
                nc.vector.memset(esm[64:128, :, :, :], 0.0)  # template keys
                nc.vector.memset(attn[64:128, :, 0, :], 0.0)  # template rows
                nc.gpsimd.memset(va[64:128, :, 2, :, :], 0.0)  # v key chunk 2
                nc.vector.memset(va[:, :, :, :, 64], 1.0)  # ones column
                nc.vector.memset(va[:, :, :, :, 65], 0.0)  # stride pad

            # ================= emission helpers =================

            def emit_xT(p):
                """DMA the pre-transposed x^T slab of pair p into SBUF."""
                xT = P["xT"].tile([128, KS, PAIR_TOK], F16, tag="xT")
                nc.sync.dma_start(
                    xT[:],
                    x_ap[:, p * PAIR_TOK : (p + 1) * PAIR_TOK].rearrange(
                        "(k p) t -> p k t", p=128
                    ),
                )
                return xT

            def emit_B(p, xT):
                """qk matmuls for pair p -> qTp (DMA) and kTpk (drain)."""
                sl = p % 2
                for fc in [0, 6, 1, 7, 2, 8, 3, 9, 4, 10, 5, 11]:
                    tg = P["pg"].tile([128, 1024], F32, tag="pg")
                    for ks in range(KS):
                        nc.tensor.matmul(
                            tg[:, 0:512],
                            lhsT=wqkv16[:, ks, fc * 128 : (fc + 1) * 128],
                            rhs=xT[:, ks, 0:512],
                            start=(ks == 0),
                            stop=(ks == KS - 1),
                        )
                    for ks in range(KS):
                        nc.tensor.matmul(
                            tg[:, 512:640],
                            lhsT=wqkv16[:, ks, fc * 128 : (fc + 1) * 128],
                            rhs=xT[:, ks, 512:640],
                            start=(ks == 0),
                            stop=(ks == KS - 1),
                        )
                    if fc < KS:  # q features -> padded per-head tiles via DMA
                        qf = P["qkfc"].tile([128, PAIR_TOK], F16, tag="qkfc")
                        nc.vector.tensor_copy(qf[:], tg[:, 0:640])
                        nc.sync.dma_start(qTp[0:64, sl, 2 * fc, :], qf[0:64, :])
                        nc.sync.dma_start(qTp[64:128, sl, 2 * fc + 1, :], qf[64:128, :])
                    else:  # k features -> packed tile directly (alternate engines)
                        if fc % 2 == 0:
                            nc.vector.tensor_copy(kTpk[:, sl, fc - KS, :], tg[:, 0:640])
                        else:
                            nc.scalar.copy(kTpk[:, sl, fc - KS, :], tg[:, 0:640])

            def make_C(g):
                """v matmuls for batch g (3 psum tiles)."""
                sl, b2 = g % 2, g % 2
                btok = (g % 2) * N
                psl = (g // 2) % 2
                pieces = []
                for ci, (off, sz) in enumerate(KT_CHUNKS):
                    holder = {}

                    def piece_a(ci=ci, off=off, sz=sz, holder=holder):
                        xT = xT_cur[g // 2]
                        tg = P["pg"].tile([128, 1024], F32, tag="pg")
                        holder["tg"] = tg
                        for ks in range(KS):
                            nc.tensor.matmul(
                                tg[:sz, 0:512],
                                lhsT=xT[:, ks, btok + off : btok + off + sz],
                                rhs=wqkv16[:, ks, 2 * C : 2 * C + 512],
                                start=(ks == 0),
                                stop=(ks == KS - 1),
                            )

                    def piece_b(ci=ci, off=off, sz=sz, holder=holder):
                        xT = xT_cur[g // 2]
                        tg = holder["tg"]
                        for ks in range(KS):
                            nc.tensor.matmul(
                                tg[:sz, 512:768],
                                lhsT=xT[:, ks, btok + off : btok + off + sz],
                                rhs=wqkv16[:, ks, 2 * C + 512 : 3 * C],
                                start=(ks == 0),
                                stop=(ks == KS - 1),
                            )
                        nc.vector.tensor_copy(
                            va[:sz, sl, ci, :, 0:64],
                            tg[:sz, 0:768].rearrange("p (h d) -> p h d", d=64),
                        )

                    pieces.append(piece_a)
                    pieces.append(piece_b)
                return pieces

            def emit_D(g, fillers):
                """Scores + exp for batch g, interleaving filler pieces."""
                sl = g % 2
                psl = (g // 2) % 2
                btok = (g % 2) * N
                fi = 0
                nf = len(fillers)
                gi = 0
                for ci, (koff, ksz) in enumerate(KT_CHUNKS):
                    for hg in range(3):
                        # two heads per matmul: both heads' padded q side by
                        # side (N=512); the packed kT chunk's parity halves
                        # each hit their own head, zeros kill cross terms
                        psc = P["ps"].tile([128, 2, 2, 256], F32, tag="ps")
                        for hp in range(2):
                            h0 = hg * 4 + hp * 2
                            nc.tensor.matmul(
                                psc[:ksz, hp, :, :],
                                lhsT=kTpk[:, psl, h0 // 2, btok + koff : btok + koff + ksz],
                                rhs=qTp[:, psl, h0 : h0 + 2, btok + 64 : btok + 320],
                                start=True,
                                stop=True,
                            )
                        nc.scalar.activation(
                            es[:ksz, sl, ci, hg * 4 : hg * 4 + 4, :],
                            psc[:ksz, :, :, :].rearrange("p a b q -> p (a b) q"),
                            mybir.ActivationFunctionType.Exp,
                            scale=0.125,
                        )
                        gi += 1
                        want = (nf * gi) // 9
                        while fi < want:
                            fillers[fi]()
                            fi += 1
                while fi < len(fillers):
                    fillers[fi]()
                    fi += 1

            def make_E(g):
                """Template scores + exp for batch g (2 pieces)."""
                sl = g % 2
                psl = (g // 2) % 2
                btok = (g % 2) * N
                holder = {}

                def mm_piece():
                    tg = P["pg"].tile([128, 1024], F32, tag="pg")
                    holder["tg"] = tg
                    for hp in range(6):
                        h0 = 2 * hp
                        nc.tensor.matmul(
                            tg[0:64, h0 * 64 : (h0 + 2) * 64],
                            lhsT=kTpk[:, psl, hp, btok : btok + 64],
                            rhs=qTp[:, psl, h0 : h0 + 2, btok : btok + 64],
                            start=True,
                            stop=True,
                        )

                def exp_piece():
                    tg = holder["tg"]
                    nc.scalar.activation(
                        esm[0:64, sl, :, :],
                        tg[0:64, 0:768].rearrange("p (h q) -> p h q", q=64),
                        mybir.ActivationFunctionType.Exp,
                        scale=0.125,
                    )

                return [mm_piece, exp_piece]

            def _normalize(tg, qsz, qg, half, sl):
                po_v = tg[:qsz, 0:510].rearrange("p (h s) -> p h s", s=SLOT)
                rcp = P["rcp"].tile([128, 8], F32, tag="rcp")
                nc.vector.reciprocal(rcp[:qsz, 0:6], po_v[:, :, 64])
                nc.vector.tensor_tensor(
                    attn[:qsz, sl, qg, half * 384 : (half + 1) * 384].rearrange(
                        "p (h d) -> p h d", d=64
                    ),
                    po_v[:, :, 0:64],
                    rcp[:qsz, 0:6, None].to_broadcast([qsz, 6, 64]),
                    mybir.AluOpType.mult,
                )

            def make_FGH(g):
                """PV + normalize + attn^T + proj for batch g (deferred)."""
                sl = g % 2
                pieces = []

                # template PV (2 pieces, one per head-half)
                tpv_pieces = []
                for half in range(2):
                    def tpv(half=half):
                        tg = P["pg"].tile([128, 1024], F32, tag="pg")
                        for j in range(6):
                            h = half * 6 + j
                            nc.tensor.matmul(
                                tg[0:64, j * SLOT : j * SLOT + 65],
                                lhsT=esm[:, sl, h, 0:64],
                                rhs=va[:, sl, 0, h, 0:65],
                                start=True,
                                stop=True,
                            )
                        _normalize(tg, 64, 0, half, sl)
                    tpv_pieces.append(tpv)

                # search PV (4 pieces: qg x half)
                spv_pieces = {}
                for qg in (1, 2):
                    for half in range(2):
                        def spv(qg=qg, half=half):
                            tg = P["pg"].tile([128, 1024], F32, tag="pg")
                            for j in range(6):
                                h = half * 6 + j
                                for ci in range(3):
                                    nc.tensor.matmul(
                                        tg[0:128, j * SLOT : j * SLOT + 65],
                                        lhsT=es[:, sl, ci, h, (qg - 1) * 128 : qg * 128],
                                        rhs=va[:, sl, ci, h, 0:65],
                                        start=(ci == 0),
                                        stop=(ci == 2),
                                    )
                            _normalize(tg, 128, qg, half, sl)
                        spv_pieces[(qg, half)] = spv

                # attn^T via regular matmuls (6 pieces)
                at_pieces = []
                for fc in range(KS):
                    def at(fc=fc):
                        tg = P["pg"].tile([128, 1024], F32, tag="pg")
                        # overlap-packed: qg0 -> 0:128 (real 0:64), qg1 -> 64:192,
                        # qg2 -> 192:320
                        for qg, dst0 in ((0, 0), (1, 64), (2, 192)):
                            nc.tensor.matmul(
                                tg[:, dst0 : dst0 + 128],
                                lhsT=attn[0:128, sl, qg, fc * 128 : (fc + 1) * 128],
                                rhs=ident16[:, 0:128],
                                start=True,
                                stop=True,
                            )
                        nc.vector.tensor_copy(attnT[:, sl, fc, 0:N], tg[:, 0:N])
                    at_pieces.append(at)

                # proj + bias + out DMA (3 pieces)
                pieces = []
                row0 = g * N
                for qc, (qoff, qsz) in enumerate(P_CHUNKS):
                    def pj(qc=qc, qoff=qoff, qsz=qsz):
                        tg = P["pg"].tile([128, 1024], F32, tag="pg")
                        for ks in range(KS):
                            nc.tensor.matmul(
                                tg[:qsz, 0:512],
                                lhsT=attnT[:, sl, ks, qoff : qoff + qsz],
                                rhs=wproj16[:, ks, 0:512],
                                start=(ks == 0),
                                stop=(ks == KS - 1),
                            )
                        for ks in range(KS):
                            nc.tensor.matmul(
                                tg[:qsz, 512:768],
                                lhsT=attnT[:, sl, ks, qoff : qoff + qsz],
                                rhs=wproj16[:, ks, 512:768],
                                start=(ks == 0),
                                stop=(ks == KS - 1),
                            )
                        ost = P["outst"].tile([128, C], F16, tag="outst")
                        nc.vector.tensor_tensor(
                            ost[:qsz, :], tg[:qsz, 0:768], bias_bc[:qsz, :],
                            mybir.AluOpType.add,
                        )
                        nc.sync.dma_start(
                            out_ap[row0 + qoff : row0 + qoff + qsz, :], ost[:qsz, :]
                        )
                    pieces.append(pj)
                pj_pieces = pieces
                # order: half-0 PV -> attnT fc 0-2 -> half-1 PV -> attnT 3-5
                # -> proj; gets attn^T/proj flowing as early as possible
                return (
                    [spv_pieces[(1, 0)], spv_pieces[(2, 0)], tpv_pieces[0]]
                    + at_pieces[0:3]
                    + [spv_pieces[(1, 1)], spv_pieces[(2, 1)], tpv_pieces[1]]
                    + at_pieces[3:6]
                    + pj_pieces
                )

            # ================= main schedule =================
            # x DMAs first (unblock PE transposes ASAP), then the weight
            # block (HBM-bound), pads last so DVE drains xT promptly.
            xT_cur = {}
            xT_cur[0] = emit_xT(0)
            emit_weight_load()
            emit_pads()

            stash = []
            for p in range(NPAIR):
                # flush only the half-0 block of FGH(2p-1) here; the rest
                # becomes D(b0) filler so scores never starve
                for pc in stash[:6]:
                    pc()
                rest = stash[6:]
                stash = []
                if p + 1 < NPAIR:
                    xT_cur[p + 1] = emit_xT(p + 1)
                emit_B(p, xT_cur[p])
                g0, g1 = 2 * p, 2 * p + 1
                for pc in make_C(g0):
                    pc()
                fill0 = rest + make_C(g1) + make_E(g0)
                emit_D(g0, fill0)
                stash0 = make_FGH(g0)
                emit_D(g1, stash0[0:6] + make_E(g1) + stash0[6:])
                stash = make_FGH(g1)
            for pc in stash:
                pc()

    nc.compile()
    return nc


@functools.cache
def _get_nc():
    return build_kernel()


def make_in_maps(x, wqkv, wproj, bias):
    x16 = x.reshape(B, N, C).astype(np.float16)
    wqkv16 = np.ascontiguousarray(wqkv.astype(np.float16))
    wproj16 = np.ascontiguousarray(wproj.astype(np.float16))
    bias = np.ascontiguousarray(bias.astype(np.float32))
    return [
        {
            "xT16": np.ascontiguousarray(
                x16[c * B_CORE : (c + 1) * B_CORE].reshape(TOK_CORE, C).T
            ),
            "W_qkv16": wqkv16,
            "W_proj16": wproj16,
            "b_proj": bias,
        }
        for c in range(NCORES)
    ]


def kernel(**inputs):
    x = np.ascontiguousarray(np.asarray(inputs["x"], dtype=np.float32))
    wqkv = np.ascontiguousarray(np.asarray(inputs["W_qkv"], dtype=np.float32))
    wproj = np.ascontiguousarray(np.asarray(inputs["W_proj"], dtype=np.float32))
    bias = np.ascontiguousarray(np.asarray(inputs["b_proj"], dtype=np.float32))
    t_h = int(inputs.get("t_h", 8))
    t_w = int(inputs.get("t_w", 8))
    assert t_h * t_w == 64, "kernel built for template length 64"
    assert x.shape == (B, N, C)

    nc = _get_nc()
    in_maps = make_in_maps(x, wqkv, wproj, bias)
    res = run_bass_kernel_spmd(nc, in_maps, core_ids=list(range(NCORES)))
    out = np.concatenate(
        [r["out"].astype(np.float32).reshape(B_CORE, N, C) for r in res.results],
        axis=0,
    )
    return out


if __name__ == "__main__":
    _get_nc()
    print("kernel_v2 built OK")

